# revision 1
# baseline (speedup 1.0000x reference)
"""Trainium2 Bass kernel for a complex-valued LSTM (nn_ComplexLSTMCell).

Math (per time step, complex arithmetic with real/imag stored split):
    z  = W x_t + R h_{t-1} + b          (complex affine, 4 gates x U units)
    i, f, o = sigmoid(z0, z1, z3);  g = tanh(z2)   (component-wise on re/im)
    c_t = f*c + i*g                      (complex elementwise products)
    h_t = o * tanh(c_t)                  (tanh applied component-wise to c_t)

Strategy: data-parallel across 8 NeuronCores (32 batch rows each).
Per core everything runs in a "z-transposed" layout [units(128 partitions),
batch(free)] so gate elementwise uses all 128 lanes:
  - x-projection zx = W x + b for a whole 64-step block is done with big
    matmuls (PE) and kept SBUF-resident in fp16.
  - per step: z = zx_t (injected into PSUM via identity-matmul) + 16
    accumulating [128,128]x[128,32] matmuls for R h.
  - gates on ScalarE (sigmoid/tanh, one table set), complex c/h updates
    as wide VectorE ops using strided APs.
  - h_t pairs are transposed back to batch-major via PE transpose and
    DMA'd out every 8 steps.
"""
import os
import numpy as np

_ABL = set(os.environ.get("KABL", "").split(","))  # timing-ablation switches

B, T, DIN, U = 256, 512, 64, 128
NCORES = 8
BL = B // NCORES          # 32 batch rows per core
TBLK = 64                 # steps per zx block
F2 = 2 * DIN              # 128: complex input features (re|im)
G8 = 8                    # gate chunks: f_r f_i i_r i_i o_r o_i g_r g_i

# gate index in reference weights: 0=i 1=f 2=g(tanh) 3=o
CHUNKS = [(1, 'r'), (1, 'i'), (0, 'r'), (0, 'i'), (3, 'r'), (3, 'i'), (2, 'r'), (2, 'i')]

_CACHE = {}


def _build_weights(kernel_real, kernel_imag, rec_real, rec_imag, bias_real, bias_imag):
    Wb = np.zeros((G8, F2, U), np.float32)       # (chunk, K=feat, M=units)
    Rb = np.zeros((2, G8, U, U), np.float32)     # (kchunk, chunk, K, M)
    bias = np.zeros((U, G8), np.float32)         # (unit, chunk)
    for c, (g, part) in enumerate(CHUNKS):
        cols = slice(g * U, (g + 1) * U)
        if part == 'r':
            Wb[c] = np.concatenate([kernel_real[:, cols], -kernel_imag[:, cols]], axis=0)
            Rb[0, c] = rec_real[:, cols]
            Rb[1, c] = -rec_imag[:, cols]
            bias[:, c] = bias_real[cols]
        else:
            Wb[c] = np.concatenate([kernel_imag[:, cols], kernel_real[:, cols]], axis=0)
            Rb[0, c] = rec_imag[:, cols]
            Rb[1, c] = rec_real[:, cols]
            bias[:, c] = bias_imag[cols]
    return Wb.astype(np.float16), Rb.astype(np.float16), bias


def _cap(tile_ap, col_offset, nest):
    """Column-strided AP: same tensor/partition dim, custom free-dim nest.

    nest: list of [step, count] in elements of the tile's free dim.
    """
    import concourse.bass as bass
    base = tile_ap[:, col_offset:col_offset + 1]
    return bass.AP(tensor=base.tensor, offset=base.offset,
                   ap=[list(base.ap[0])] + [list(p) for p in nest])


def _build_program(t_total=T, tblk=TBLK):
    import concourse.bacc as bacc
    import concourse.tile as tile
    from concourse import mybir
    from contextlib import ExitStack

    f16 = mybir.dt.float16
    f32 = mybir.dt.float32
    nblk = t_total // tblk
    Sig = mybir.ActivationFunctionType.Sigmoid
    Tanh = mybir.ActivationFunctionType.Tanh
    Copy = mybir.ActivationFunctionType.Copy
    Ident = mybir.ActivationFunctionType.Identity

    nc = bacc.Bacc("TRN2", target_bir_lowering=False, debug=False)

    x_d = nc.dram_tensor("x", [BL, t_total, F2], f32, kind="ExternalInput").ap()
    h0_d = nc.dram_tensor("h0", [BL, 2 * U], f32, kind="ExternalInput").ap()
    c0_d = nc.dram_tensor("c0", [BL, 2 * U], f32, kind="ExternalInput").ap()
    wb_d = nc.dram_tensor("wb", [G8, F2, U], f16, kind="ExternalInput").ap()
    rb_d = nc.dram_tensor("rb", [2, G8, U, U], f16, kind="ExternalInput").ap()
    bias_d = nc.dram_tensor("bias", [U, G8], f32, kind="ExternalInput").ap()
    id16_d = nc.dram_tensor("id16", [128, 128], f16, kind="ExternalInput").ap()
    id32_d = nc.dram_tensor("id32", [128, 128], f32, kind="ExternalInput").ap()
    out_d = nc.dram_tensor("out", [BL, t_total, 2 * U], f32, kind="ExternalOutput").ap()

    with tile.TileContext(nc) as tc, ExitStack() as ctx:
        consts = ctx.enter_context(tc.tile_pool(name="consts", bufs=1))
        state = ctx.enter_context(tc.tile_pool(name="state", bufs=1))
        xnatp = ctx.enter_context(tc.tile_pool(name="xnat", bufs=2))
        xtp = ctx.enter_context(tc.tile_pool(name="xTp", bufs=2))
        stagep = ctx.enter_context(tc.tile_pool(name="stagep", bufs=2))
        zsig_pool = ctx.enter_context(tc.tile_pool(name="zsig", bufs=2, space="PSUM"))
        zg_pool = ctx.enter_context(tc.tile_pool(name="zgp", bufs=2, space="PSUM"))
        htp_pool = ctx.enter_context(tc.tile_pool(name="htp", bufs=1, space="PSUM"))
        xps_pool = ctx.enter_context(tc.tile_pool(name="xps", bufs=2, space="PSUM"))

        # ---- constants ----
        W_sb = consts.tile([128, G8, U], f16)
        R_sb = consts.tile([128, 2, G8, U], f16)
        bias_sb = consts.tile([128, G8], f32)
        id16 = consts.tile([128, 128], f16)
        id32 = consts.tile([128, 128], f32)
        nc.sync.dma_start(out=W_sb, in_=wb_d.rearrange("c K m -> K c m"))
        nc.sync.dma_start(out=R_sb, in_=rb_d.rearrange("k c K m -> K k c m"))
        nc.sync.dma_start(out=bias_sb, in_=bias_d)
        nc.sync.dma_start(out=id16, in_=id16_d)
        nc.sync.dma_start(out=id32, in_=id32_d)

        # ---- state tiles ----
        CG = state.tile([128, 128], f16)      # [cr|ci|g_r|g_i]
        Hpair = state.tile([128, 128], f16)   # [hr_e|hi_e|hr_o|hi_o]
        A = state.tile([128, 6 * BL], f16)    # sigmoid outs [f_r f_i i_r i_i o_r o_i]
        Mt = state.tile([128, 256], f16)
        Sst = state.tile([128, 128], f16)
        TC = state.tile([128, 64], f16)
        zx_buf = state.tile([128, 2, G8, tblk * BL], f16)

        # ---- initial state: transpose h0/c0 into [unit, batch] layout ----
        hc_sb = state.tile([BL, 2 * (2 * U)], f32)
        nc.sync.dma_start(out=hc_sb[:, 0:2 * U], in_=h0_d)
        nc.sync.dma_start(out=hc_sb[:, 2 * U:], in_=c0_d)
        init_ps = htp_pool.tile([128, 128], f32, name="init_ps", tag="htp")
        for j in range(4):  # hr hi cr ci
            nc.tensor.transpose(init_ps[:, j * 32:(j + 1) * 32],
                                hc_sb[:, j * U:(j + 1) * U], id32[:BL, :BL])
        # h0 -> odd-parity slot (step 0 reads rpar=1), c0 -> CG[:, 0:64]
        nc.scalar.activation(Hpair[:, 64:128], init_ps[:, 0:64], Copy)
        nc.scalar.activation(CG[:, 0:64], init_ps[:, 64:128], Copy)

        # ---- x-phase emitters ----
        def emit_xphase_dma(blk):
            # x_nat rows = (t%4, b), tiles along t//4: 4 strided DMAs
            x_nat = xnatp.tile([128, tblk // 4, F2], f32, name="x_nat", tag="x_nat")
            t0 = blk * tblk
            for tp in range(4):
                nc.sync.dma_start(
                    out=x_nat[tp * BL:(tp + 1) * BL, :, :],
                    in_=x_d[:, t0 + tp:t0 + tblk:4, :])
            xT = xtp.tile([128, tblk // 4, F2], f16, name="xT", tag="xT")
            return x_nat, xT

        def emit_xphase_transpose(x_nat, xT, i):
            # transpose 4 [128,128] chunks into one PSUM bank, cast to fp16
            xt_ps = xps_pool.tile([128, 512], f32, name="xt_ps", tag="xps")
            for j in range(4):
                nc.tensor.transpose(xt_ps[:, j * 128:(j + 1) * 128],
                                    x_nat[:, 4 * i + j, :], id32)
            nc.vector.tensor_copy(xT[:, 4 * i:4 * i + 4, :], xt_ps)

        def emit_xphase_mm(xT, blk, c, j):
            # zx[c, j*512:(j+1)*512] for block blk, cast + bias to fp16 SBUF
            bb = blk % 2
            zx_ps = xps_pool.tile([128, 512], f32, name="zx_ps", tag="xps")
            nc.tensor.matmul(zx_ps, lhsT=W_sb[:, c, :], rhs=xT[:, 4 * j:4 * j + 4, :],
                             start=True, stop=True)
            dst = zx_buf[:, bb, c, j * 512:(j + 1) * 512]
            if (c + j) % 2 == 0:
                nc.scalar.activation(dst, zx_ps, Ident, bias=bias_sb[:, c:c + 1])
            else:
                nc.vector.tensor_scalar_add(dst, zx_ps, bias_sb[:, c:c + 1])

        # ---- one recurrence step ----
        def emit_step(t):
            blk = t // tblk
            tl = t % tblk
            bb = blk % 2
            par = t % 2
            rpar = (t + 1) % 2  # parity slot holding h_{t-1}

            zs = zsig_pool.tile([128, 6 * BL], f32, name="zs", tag="zs")
            zg = zg_pool.tile([128, 2 * BL], f32, name="zg", tag="zg")

            # --- PE: z = zx_t + R h ---
            if "idcontig" in _ABL:
                zx_s = zx_buf[:, bb, 0, 0:6 * BL]
                zx_g = zx_buf[:, bb, 0, 0:2 * BL]
            else:
                zx_s = zx_buf[:, bb, 0:6, tl * BL:(tl + 1) * BL]
                zx_g = zx_buf[:, bb, 6:8, tl * BL:(tl + 1) * BL]
            nc.tensor.matmul(zs, lhsT=id16, rhs=zx_s, start=True, stop=False)
            for k in range(2):
                hk = Hpair[:, rpar * 64 + k * BL: rpar * 64 + (k + 1) * BL]
                for c in range(6):
                    nc.tensor.matmul(zs[:, c * BL:(c + 1) * BL], lhsT=R_sb[:, k, c, :],
                                     rhs=hk, start=False, stop=(k == 1 and c == 5))
            nc.tensor.matmul(zg, lhsT=id16, rhs=zx_g, start=True, stop=False)
            for k in range(2):
                hk = Hpair[:, rpar * 64 + k * BL: rpar * 64 + (k + 1) * BL]
                for c in range(6, 8):
                    nc.tensor.matmul(zg[:, (c - 6) * BL:(c - 5) * BL], lhsT=R_sb[:, k, c, :],
                                     rhs=hk, start=False, stop=(k == 1 and c == 7))

            # --- ACT: gates ---
            TanhA = Sig if "notanh" in _ABL else Tanh
            nc.scalar.activation(A, zs, Sig)
            nc.scalar.activation(CG[:, 64:128], zg, TanhA)

            # --- DVE: complex c update ---
            # M1 = [f_r f_i i_r i_i] * [cr ci g_r g_i]
            nc.vector.tensor_mul(Mt[:, 0:128], A[:, 0:128], CG[:, 0:128])
            # M2 = [f_r f_i i_r i_i] * [ci cr g_i g_r]
            if "dvecontig" in _ABL:
                nc.vector.tensor_mul(Mt[:, 128:256], A[:, 0:128], CG[:, 0:128])
            else:
                nc.vector.tensor_mul(Mt[:, 128:256], A[:, 0:128],
                                     _cap(CG, 32, [[64, 2], [-32, 2], [1, 32]]))
            # S1 = [f_r*cr - f_i*ci | i_r*g_r - i_i*g_i]
            if "dvecontig" in _ABL:
                nc.vector.tensor_sub(Sst[:, 0:64], Mt[:, 0:64], Mt[:, 64:128])
            else:
                nc.vector.tensor_sub(Sst[:, 0:64],
                                     _cap(Mt, 0, [[64, 2], [1, 32]]),
                                     _cap(Mt, 32, [[64, 2], [1, 32]]))
            # S2 = f_r*ci + f_i*cr ; S3 = i_r*g_i - i_i*g_r
            nc.vector.tensor_add(Sst[:, 64:96], Mt[:, 128:160], Mt[:, 160:192])
            nc.vector.tensor_sub(Sst[:, 96:128], Mt[:, 192:224], Mt[:, 224:256])
            # C = [S1a+S1b | S2+S3]
            if "dvecontig" in _ABL:
                nc.vector.tensor_add(CG[:, 0:64], Sst[:, 0:64], Sst[:, 64:128])
            else:
                nc.vector.tensor_add(CG[:, 0:64],
                                     _cap(Sst, 0, [[64, 2], [1, 32]]),
                                     _cap(Sst, 32, [[64, 2], [1, 32]]))

            # --- ACT: tanh of c ---
            nc.scalar.activation(TC, CG[:, 0:64], TanhA)

            # --- DVE: h = o * tanh_c (complex) ---
            nc.vector.tensor_mul(Mt[:, 0:64], A[:, 128:192], TC)
            if "dvecontig" in _ABL:
                nc.vector.tensor_mul(Mt[:, 64:128], A[:, 128:192], TC)
            else:
                nc.vector.tensor_mul(Mt[:, 64:128], A[:, 128:192],
                                     _cap(TC, 32, [[-32, 2], [1, 32]]))
            # hr = o_r*tcr - o_i*tci ; hi = o_r*tci - o_i*tcr  (both minus -> 1 op)
            if "dvecontig" in _ABL:
                nc.vector.tensor_sub(Hpair[:, par * 64: par * 64 + 64],
                                     Mt[:, 0:64], Mt[:, 64:128])
            else:
                nc.vector.tensor_sub(Hpair[:, par * 64: par * 64 + 64],
                                     _cap(Mt, 0, [[64, 2], [1, 32]]),
                                     _cap(Mt, 32, [[64, 2], [1, 32]]))

        # ---- output staging ----
        def emit_hout(t, stage_tile):
            # after odd step t: transpose (t-1,t) h pair into stage col (t//2)%4
            jp = (t // 2) % 4
            tp_ps = htp_pool.tile([128, 128], f16, name="tp_ps", tag="htp")
            nc.tensor.transpose(tp_ps, Hpair, id16)
            nc.scalar.activation(stage_tile[:, jp, :], tp_ps, Copy)

        def emit_hout_dma(t, stage_tile):
            # after step t (t%8==7): DMA stage -> out[t-7 .. t].
            # stage partition = (tpar, half, b); 4 DMAs, one per (tpar, half).
            t0 = t - 7
            for tpar in range(2):
                for h in range(2):
                    p0 = tpar * 64 + h * 32
                    nc.sync.dma_start(
                        out=out_d[:, t0 + tpar:t0 + 8:2, h * U:(h + 1) * U],
                        in_=stage_tile[p0:p0 + 32, :, :])

        # ---- prologue: x-phase for block 0 ----
        x_nat, xT = emit_xphase_dma(0)
        for i in range(tblk // 16):
            emit_xphase_transpose(x_nat, xT, i)
        for c in range(8):
            for j in range(tblk // 16):
                emit_xphase_mm(xT, 0, c, j)

        # ---- main loop (fully unrolled) ----
        stage_tile = None
        for blk in range(nblk):
            nxt = blk + 1
            xph = []
            if nxt < nblk:
                x_nat, xT = emit_xphase_dma(nxt)
                xph += [('t', i) for i in range(tblk // 16)]
                xph += [('m', c, j) for c in range(8) for j in range(tblk // 16)]
            for tl in range(tblk):
                t = blk * tblk + tl
                if t % 8 == 0:
                    stage_tile = stagep.tile([128, 4, 128], f32,
                                             name="stage", tag="stage")
                emit_step(t)
                if t % 2 == 1:
                    emit_hout(t, stage_tile)
                if t % 8 == 7:
                    emit_hout_dma(t, stage_tile)
                # spread next-block x-phase work across this block's steps
                want_done = (tl + 1) * (len(xph) + 1) // tblk if xph else 0
                while xph and len(xph) > (tblk - 1 - tl):
                    op = xph.pop(0)
                    if op[0] == 't':
                        emit_xphase_transpose(x_nat, xT, op[1])
                    else:
                        emit_xphase_mm(xT, nxt, op[1], op[2])

    nc.compile()
    return nc


def _get_program(t_total=T, tblk=TBLK):
    key = (t_total, tblk)
    if key not in _CACHE:
        _CACHE[key] = _build_program(t_total, tblk)
    return _CACHE[key]


def kernel(x, h0, c0, kernel_real, kernel_imag,
           recurrent_kernel_real, recurrent_kernel_imag,
           bias_real, bias_imag, _t_total=T, _tblk=TBLK):
    from concourse import bass_utils

    x = np.asarray(x, np.float32)
    h0 = np.asarray(h0, np.float32)
    c0 = np.asarray(c0, np.float32)
    Wb, Rb, bias = _build_weights(np.asarray(kernel_real, np.float32),
                                  np.asarray(kernel_imag, np.float32),
                                  np.asarray(recurrent_kernel_real, np.float32),
                                  np.asarray(recurrent_kernel_imag, np.float32),
                                  np.asarray(bias_real, np.float32),
                                  np.asarray(bias_imag, np.float32))
    id16 = np.eye(128, dtype=np.float16)
    id32 = np.eye(128, dtype=np.float32)

    nc = _get_program(_t_total, _tblk)
    in_maps = []
    for i in range(NCORES):
        sl = slice(i * BL, (i + 1) * BL)
        in_maps.append({
            "x": np.ascontiguousarray(x[sl]),
            "h0": np.ascontiguousarray(h0[sl]),
            "c0": np.ascontiguousarray(c0[sl]),
            "wb": Wb, "rb": Rb, "bias": bias,
            "id16": id16, "id32": id32,
        })
    res = bass_utils.run_bass_kernel_spmd(nc, in_maps, core_ids=list(range(NCORES)))
    out = np.concatenate([res.results[i]["out"] for i in range(NCORES)], axis=0)
    return out.astype(np.float32)


if __name__ == "__main__":
    nc = _get_program()
    print("program built OK")



# revision 6
# speedup vs baseline: 14.4420x; 14.4420x over previous
"""Trainium2 Bass kernel for a complex-valued LSTM (nn_ComplexLSTMCell).

Math (per time step, complex arithmetic with real/imag stored split):
    z  = W x_t + R h_{t-1} + b          (complex affine, 4 gates x U units)
    i, f, o = sigmoid(z0, z1, z3);  g = tanh(z2)   (component-wise on re/im)
    c_t = f*c + i*g                      (complex elementwise products)
    h_t = o * tanh(c_t)                  (tanh applied component-wise to c_t)

Strategy: data-parallel across 8 NeuronCores (32 batch rows each).
Per core everything runs in a "z-transposed" layout [units(128 partitions),
batch(free)] so gate elementwise uses all 128 lanes:
  - x-projection zx = W x + b for a whole 64-step block is done with big
    matmuls (PE) and kept SBUF-resident in fp16.
  - per step: z = zx_t (injected into PSUM via identity-matmul) + 16
    accumulating [128,128]x[128,32] matmuls for R h.
  - gates on ScalarE (sigmoid/tanh, one table set), complex c/h updates
    as wide VectorE ops using strided APs.
  - h_t pairs are transposed back to batch-major via PE transpose and
    DMA'd out every 8 steps.

Host<->device path: x is shipped fp16 (the kernel rounded it to fp16 for
the matmuls anyway) and the output is produced fp16 on device and widened
to fp32 on the host; the PJRT executable, device-resident weights, and the
output-donation placeholder buffer are all built once and cached so warm
calls do no retracing/recompiling and transfer only x (in) and out (back).
"""
import os
import numpy as np

B, T, DIN, U = 256, 512, 64, 128
NCORES = 8
BL = B // NCORES          # 32 batch rows per core
TBLK = 64                 # steps per zx block
F2 = 2 * DIN              # 128: complex input features (re|im)
G8 = 8                    # gate chunks: f_r f_i i_r i_i o_r o_i g_r g_i

# gate index in reference weights: 0=i 1=f 2=g(tanh) 3=o
CHUNKS = [(1, 'r'), (1, 'i'), (0, 'r'), (0, 'i'), (3, 'r'), (3, 'i'), (2, 'r'), (2, 'i')]

_CACHE = {}


def _build_weights(kernel_real, kernel_imag, rec_real, rec_imag, bias_real, bias_imag):
    Wb = np.zeros((G8, F2, U), np.float32)       # (chunk, K=feat, M=units)
    Rb = np.zeros((2, G8, U, U), np.float32)     # (kchunk, chunk, K, M)
    bias = np.zeros((U, G8), np.float32)         # (unit, chunk)
    for c, (g, part) in enumerate(CHUNKS):
        cols = slice(g * U, (g + 1) * U)
        if part == 'r':
            Wb[c] = np.concatenate([kernel_real[:, cols], -kernel_imag[:, cols]], axis=0)
            Rb[0, c] = rec_real[:, cols]
            Rb[1, c] = -rec_imag[:, cols]
            bias[:, c] = bias_real[cols]
        else:
            Wb[c] = np.concatenate([kernel_imag[:, cols], kernel_real[:, cols]], axis=0)
            Rb[0, c] = rec_imag[:, cols]
            Rb[1, c] = rec_real[:, cols]
            bias[:, c] = bias_imag[cols]
    return Wb.astype(np.float16), Rb.astype(np.float16), bias


def _cap(tile_ap, col_offset, nest):
    """Column-strided AP: same tensor/partition dim, custom free-dim nest.

    nest: list of [step, count] in elements of the tile's free dim.
    """
    import concourse.bass as bass
    base = tile_ap[:, col_offset:col_offset + 1]
    return bass.AP(tensor=base.tensor, offset=base.offset,
                   ap=[list(base.ap[0])] + [list(p) for p in nest])


def _build_program(t_total=T, tblk=TBLK):
    import concourse.bacc as bacc
    import concourse.tile as tile
    from concourse import mybir
    from contextlib import ExitStack

    f16 = mybir.dt.float16
    f32 = mybir.dt.float32
    nblk = t_total // tblk
    Sig = mybir.ActivationFunctionType.Sigmoid
    Tanh = mybir.ActivationFunctionType.Tanh
    Copy = mybir.ActivationFunctionType.Copy
    Ident = mybir.ActivationFunctionType.Identity

    nc = bacc.Bacc("TRN2", target_bir_lowering=False, debug=False)

    x_d = nc.dram_tensor("x", [BL, t_total, F2], f16, kind="ExternalInput").ap()
    h0_d = nc.dram_tensor("h0", [BL, 2 * U], f32, kind="ExternalInput").ap()
    c0_d = nc.dram_tensor("c0", [BL, 2 * U], f32, kind="ExternalInput").ap()
    wb_d = nc.dram_tensor("wb", [G8, F2, U], f16, kind="ExternalInput").ap()
    rb_d = nc.dram_tensor("rb", [2, G8, U, U], f16, kind="ExternalInput").ap()
    bias_d = nc.dram_tensor("bias", [U, G8], f32, kind="ExternalInput").ap()
    id16_d = nc.dram_tensor("id16", [128, 128], f16, kind="ExternalInput").ap()
    id32_d = nc.dram_tensor("id32", [128, 128], f32, kind="ExternalInput").ap()
    out_d = nc.dram_tensor("out", [BL, t_total, 2 * U], f16, kind="ExternalOutput").ap()

    with tile.TileContext(nc) as tc, ExitStack() as ctx:
        consts = ctx.enter_context(tc.tile_pool(name="consts", bufs=1))
        state = ctx.enter_context(tc.tile_pool(name="state", bufs=1))
        xnatp = ctx.enter_context(tc.tile_pool(name="xnat", bufs=2))
        xtp = ctx.enter_context(tc.tile_pool(name="xTp", bufs=2))
        stagep = ctx.enter_context(tc.tile_pool(name="stagep", bufs=2))
        zsig_pool = ctx.enter_context(tc.tile_pool(name="zsig", bufs=2, space="PSUM"))
        zg_pool = ctx.enter_context(tc.tile_pool(name="zgp", bufs=2, space="PSUM"))
        htp_pool = ctx.enter_context(tc.tile_pool(name="htp", bufs=1, space="PSUM"))
        xps_pool = ctx.enter_context(tc.tile_pool(name="xps", bufs=2, space="PSUM"))

        # ---- constants ----
        W_sb = consts.tile([128, G8, U], f16)
        R_sb = consts.tile([128, 2, G8, U], f16)
        bias_sb = consts.tile([128, G8], f32)
        id16 = consts.tile([128, 128], f16)
        id32 = consts.tile([128, 128], f32)
        nc.sync.dma_start(out=W_sb, in_=wb_d.rearrange("c K m -> K c m"))
        nc.sync.dma_start(out=R_sb, in_=rb_d.rearrange("k c K m -> K k c m"))
        nc.sync.dma_start(out=bias_sb, in_=bias_d)
        nc.sync.dma_start(out=id16, in_=id16_d)
        nc.sync.dma_start(out=id32, in_=id32_d)

        # ---- state tiles ----
        CG = state.tile([128, 128], f16)      # [cr|ci|g_r|g_i]
        Hpair = state.tile([128, 128], f16)   # [hr_e|hi_e|hr_o|hi_o]
        A = state.tile([128, 6 * BL], f16)    # sigmoid outs [f_r f_i i_r i_i o_r o_i]
        Mt = state.tile([128, 256], f16)
        Sst = state.tile([128, 128], f16)
        TC = state.tile([128, 64], f16)
        zx_buf = state.tile([128, 2, G8, tblk * BL], f16)

        # ---- initial state: transpose h0/c0 into [unit, batch] layout ----
        hc_sb = state.tile([BL, 2 * (2 * U)], f32)
        nc.sync.dma_start(out=hc_sb[:, 0:2 * U], in_=h0_d)
        nc.sync.dma_start(out=hc_sb[:, 2 * U:], in_=c0_d)
        init_ps = htp_pool.tile([128, 128], f32, name="init_ps", tag="htp")
        for j in range(4):  # hr hi cr ci
            nc.tensor.transpose(init_ps[:, j * 32:(j + 1) * 32],
                                hc_sb[:, j * U:(j + 1) * U], id32[:BL, :BL])
        # h0 -> odd-parity slot (step 0 reads rpar=1), c0 -> CG[:, 0:64]
        nc.scalar.activation(Hpair[:, 64:128], init_ps[:, 0:64], Copy)
        nc.scalar.activation(CG[:, 0:64], init_ps[:, 64:128], Copy)

        # ---- x-phase emitters ----
        def emit_xphase_dma(blk):
            # x_nat rows = (t%4, b), tiles along t//4: 4 strided DMAs
            x_nat = xnatp.tile([128, tblk // 4, F2], f16, name="x_nat", tag="x_nat")
            t0 = blk * tblk
            for tp in range(4):
                nc.sync.dma_start(
                    out=x_nat[tp * BL:(tp + 1) * BL, :, :],
                    in_=x_d[:, t0 + tp:t0 + tblk:4, :])
            xT = xtp.tile([128, tblk // 4, F2], f16, name="xT", tag="xT")
            return x_nat, xT

        def emit_xphase_transpose(x_nat, xT, i):
            # transpose 4 [128,128] chunks into one PSUM bank, cast to fp16
            xt_ps = xps_pool.tile([128, 512], f16, name="xt_ps", tag="xps")
            for j in range(4):
                nc.tensor.transpose(xt_ps[:, j * 128:(j + 1) * 128],
                                    x_nat[:, 4 * i + j, :], id16)
            nc.vector.tensor_copy(xT[:, 4 * i:4 * i + 4, :], xt_ps)

        def emit_xphase_mm(xT, blk, c, j):
            # zx[c, j*512:(j+1)*512] for block blk, cast + bias to fp16 SBUF
            bb = blk % 2
            zx_ps = xps_pool.tile([128, 512], f32, name="zx_ps", tag="xps")
            nc.tensor.matmul(zx_ps, lhsT=W_sb[:, c, :], rhs=xT[:, 4 * j:4 * j + 4, :],
                             start=True, stop=True)
            dst = zx_buf[:, bb, c, j * 512:(j + 1) * 512]
            if (c + j) % 2 == 0:
                nc.scalar.activation(dst, zx_ps, Ident, bias=bias_sb[:, c:c + 1])
            else:
                nc.vector.tensor_scalar_add(dst, zx_ps, bias_sb[:, c:c + 1])

        # ---- one recurrence step ----
        def emit_step(t):
            blk = t // tblk
            tl = t % tblk
            bb = blk % 2
            par = t % 2
            rpar = (t + 1) % 2  # parity slot holding h_{t-1}

            zs = zsig_pool.tile([128, 6 * BL], f32, name="zs", tag="zs")
            zg = zg_pool.tile([128, 2 * BL], f32, name="zg", tag="zg")

            # --- PE: z = zx_t + R h ---
            zx_s = zx_buf[:, bb, 0:6, tl * BL:(tl + 1) * BL]
            zx_g = zx_buf[:, bb, 6:8, tl * BL:(tl + 1) * BL]
            nc.tensor.matmul(zs, lhsT=id16, rhs=zx_s, start=True, stop=False)
            for k in range(2):
                hk = Hpair[:, rpar * 64 + k * BL: rpar * 64 + (k + 1) * BL]
                for c in range(6):
                    nc.tensor.matmul(zs[:, c * BL:(c + 1) * BL], lhsT=R_sb[:, k, c, :],
                                     rhs=hk, start=False, stop=(k == 1 and c == 5))
            nc.tensor.matmul(zg, lhsT=id16, rhs=zx_g, start=True, stop=False)
            for k in range(2):
                hk = Hpair[:, rpar * 64 + k * BL: rpar * 64 + (k + 1) * BL]
                for c in range(6, 8):
                    nc.tensor.matmul(zg[:, (c - 6) * BL:(c - 5) * BL], lhsT=R_sb[:, k, c, :],
                                     rhs=hk, start=False, stop=(k == 1 and c == 7))

            # --- ACT: gates ---
            nc.scalar.activation(A, zs, Sig)
            nc.scalar.activation(CG[:, 64:128], zg, Tanh)

            # --- DVE: complex c update ---
            # M1 = [f_r f_i i_r i_i] * [cr ci g_r g_i]
            nc.vector.tensor_mul(Mt[:, 0:128], A[:, 0:128], CG[:, 0:128])
            # M2 = [f_r f_i i_r i_i] * [ci cr g_i g_r]
            nc.vector.tensor_mul(Mt[:, 128:256], A[:, 0:128],
                                 _cap(CG, 32, [[64, 2], [-32, 2], [1, 32]]))
            # S1 = [f_r*cr - f_i*ci | i_r*g_r - i_i*g_i]
            nc.vector.tensor_sub(Sst[:, 0:64],
                                 _cap(Mt, 0, [[64, 2], [1, 32]]),
                                 _cap(Mt, 32, [[64, 2], [1, 32]]))
            # S2 = f_r*ci + f_i*cr ; S3 = i_r*g_i - i_i*g_r
            nc.vector.tensor_add(Sst[:, 64:96], Mt[:, 128:160], Mt[:, 160:192])
            nc.vector.tensor_sub(Sst[:, 96:128], Mt[:, 192:224], Mt[:, 224:256])
            # C = [S1a+S1b | S2+S3]
            nc.vector.tensor_add(CG[:, 0:64],
                                 _cap(Sst, 0, [[64, 2], [1, 32]]),
                                 _cap(Sst, 32, [[64, 2], [1, 32]]))

            # --- ACT: tanh of c ---
            nc.scalar.activation(TC, CG[:, 0:64], Tanh)

            # --- DVE: h = o * tanh_c (complex) ---
            nc.vector.tensor_mul(Mt[:, 0:64], A[:, 128:192], TC)
            nc.vector.tensor_mul(Mt[:, 64:128], A[:, 128:192],
                                 _cap(TC, 32, [[-32, 2], [1, 32]]))
            # hr = o_r*tcr - o_i*tci ; hi = o_r*tci - o_i*tcr  (both minus -> 1 op)
            nc.vector.tensor_sub(Hpair[:, par * 64: par * 64 + 64],
                                 _cap(Mt, 0, [[64, 2], [1, 32]]),
                                 _cap(Mt, 32, [[64, 2], [1, 32]]))

        # ---- output staging ----
        def emit_hout(t, stage_tile):
            # after odd step t: transpose (t-1,t) h pair into stage col (t//2)%4
            jp = (t // 2) % 4
            tp_ps = htp_pool.tile([128, 128], f16, name="tp_ps", tag="htp")
            nc.tensor.transpose(tp_ps, Hpair, id16)
            nc.scalar.activation(stage_tile[:, jp, :], tp_ps, Copy)

        def emit_hout_dma(t, stage_tile):
            # after step t (t%8==7): DMA stage -> out[t-7 .. t].
            # stage partition = (tpar, half, b); 4 DMAs, one per (tpar, half).
            t0 = t - 7
            for tpar in range(2):
                for h in range(2):
                    p0 = tpar * 64 + h * 32
                    nc.sync.dma_start(
                        out=out_d[:, t0 + tpar:t0 + 8:2, h * U:(h + 1) * U],
                        in_=stage_tile[p0:p0 + 32, :, :])

        # ---- prologue: x-phase for block 0 ----
        x_nat, xT = emit_xphase_dma(0)
        for i in range(tblk // 16):
            emit_xphase_transpose(x_nat, xT, i)
        for c in range(8):
            for j in range(tblk // 16):
                emit_xphase_mm(xT, 0, c, j)

        # ---- main loop (fully unrolled) ----
        stage_tile = None
        for blk in range(nblk):
            nxt = blk + 1
            xph = []
            if nxt < nblk:
                x_nat, xT = emit_xphase_dma(nxt)
                xph += [('t', i) for i in range(tblk // 16)]
                xph += [('m', c, j) for c in range(8) for j in range(tblk // 16)]
            for tl in range(tblk):
                t = blk * tblk + tl
                if t % 8 == 0:
                    stage_tile = stagep.tile([128, 4, 128], f16,
                                             name="stage", tag="stage")
                emit_step(t)
                if t % 2 == 1:
                    emit_hout(t, stage_tile)
                if t % 8 == 7:
                    emit_hout_dma(t, stage_tile)
                # spread next-block x-phase work across this block's steps
                while xph and len(xph) > (tblk - 1 - tl):
                    op = xph.pop(0)
                    if op[0] == 't':
                        emit_xphase_transpose(x_nat, xT, op[1])
                    else:
                        emit_xphase_mm(xT, nxt, op[1], op[2])

    nc.compile()
    return nc


def _get_program(t_total=T, tblk=TBLK):
    key = (t_total, tblk)
    if key not in _CACHE:
        _CACHE[key] = _build_program(t_total, tblk)
    return _CACHE[key]


# ---------------------------------------------------------------------------
# Cached PJRT execution path.
#
# bass_utils.run_bass_kernel_spmd -> bass2jax.run_bass_via_pjrt builds a fresh
# jax.jit closure on every call, so warm calls re-trace, re-serialize the BIR
# into the HLO, and re-run XLA/NEFF compilation; it also ships host-built zero
# output buffers (full fp32 output size) over the relay each call. This class
# replicates its lowering exactly but builds the jitted executable once, keeps
# weights/constants device-resident, and keeps a device-side placeholder for
# the output operand (the kernel writes every output element, so the
# placeholder contents are never observed and it can be reused, undonated).
# ---------------------------------------------------------------------------
class _Runner:
    def __init__(self, nc, n_cores):
        import jax
        import jax.numpy as jnp
        from jax.sharding import Mesh, NamedSharding, PartitionSpec
        from jax.experimental.shard_map import shard_map
        from concourse import bass2jax, mybir

        bass2jax.install_neuronx_cc_hook()
        assert nc.dbg_addr is None, "build the program with debug=False"

        partition_name = (nc.partition_id_tensor.name
                          if nc.partition_id_tensor else None)
        in_names, out_names, out_avals = [], [], []
        for alloc in nc.m.functions[0].allocations:
            if not isinstance(alloc, mybir.MemoryLocationSet):
                continue
            name = alloc.memorylocations[0].name
            if alloc.kind == "ExternalInput":
                if name != partition_name:
                    in_names.append(name)
            elif alloc.kind == "ExternalOutput":
                shape = tuple(alloc.tensor_shape)
                dtype = mybir.dt.np(alloc.dtype)
                out_names.append(name)
                out_avals.append(jax.core.ShapedArray(shape, dtype))
        self.param_names = list(in_names)
        in_names = in_names + out_names
        if partition_name is not None:
            in_names.append(partition_name)

        def _body(*args):
            operands = list(args)
            if partition_name is not None:
                operands.append(bass2jax.partition_id_tensor())
            outs = bass2jax._bass_exec_p.bind(
                *operands,
                out_avals=tuple(out_avals),
                in_names=tuple(in_names),
                out_names=tuple(out_names),
                lowering_input_output_aliases=(),
                sim_require_finite=True,
                sim_require_nnan=True,
                nc=nc,
            )
            return tuple(outs)

        devices = jax.devices()[:n_cores]
        assert len(devices) == n_cores
        mesh = Mesh(np.asarray(devices), ("core",))
        self.sharding = NamedSharding(mesh, PartitionSpec("core"))
        n_args = len(self.param_names) + len(out_names)
        self.fn = jax.jit(
            shard_map(_body, mesh=mesh,
                      in_specs=(PartitionSpec("core"),) * n_args,
                      out_specs=(PartitionSpec("core"),) * len(out_names),
                      check_rep=False),
            keep_unused=True,
        )
        # device-side placeholder for each output operand (never read back)
        self._outbuf_fn = jax.jit(
            lambda: tuple(jnp.zeros((n_cores * a.shape[0],) + a.shape[1:], a.dtype)
                          for a in out_avals),
            out_shardings=(self.sharding,) * len(out_avals),
        )
        self._outbufs = None
        self._jax = jax
        self._const_host = {}    # name -> host array last uploaded
        self._const_dev = {}     # name -> device array

    def put(self, arr):
        """Async upload of a full (n_cores*d0, ...) host array, sharded on axis 0."""
        return self._jax.device_put(arr, self.sharding)

    def put_const(self, name, arr):
        """Device-cached upload: re-uploads only if contents changed."""
        prev = self._const_host.get(name)
        if prev is not None and prev.shape == arr.shape and np.array_equal(prev, arr):
            return self._const_dev[name]
        dev = self.put(arr)
        self._const_host[name] = arr
        self._const_dev[name] = dev
        return dev

    def run(self, arg_map):
        if self._outbufs is None:
            self._outbufs = self._outbuf_fn()
        args = [arg_map[n] for n in self.param_names]
        return self.fn(*args, *self._outbufs)


_RUNNERS = {}


def _get_runner(t_total=T, tblk=TBLK):
    key = (t_total, tblk)
    if key not in _RUNNERS:
        _RUNNERS[key] = _Runner(_get_program(t_total, tblk), NCORES)
    return _RUNNERS[key]


def _tile_cores(a):
    """Replicate a per-core array n_cores times along axis 0."""
    return np.broadcast_to(a, (NCORES,) + a.shape).reshape(
        (NCORES * a.shape[0],) + a.shape[1:])


def kernel(x, h0, c0, kernel_real, kernel_imag,
           recurrent_kernel_real, recurrent_kernel_imag,
           bias_real, bias_imag, _t_total=T, _tblk=TBLK):
    x = np.asarray(x)
    r = _get_runner(_t_total, _tblk)

    # start the big x upload first so it overlaps the host-side prep below
    x16 = x.astype(np.float16) if x.dtype != np.float16 else x
    x_dev = r.put(np.ascontiguousarray(x16))

    Wb, Rb, bias = _build_weights(np.asarray(kernel_real, np.float32),
                                  np.asarray(kernel_imag, np.float32),
                                  np.asarray(recurrent_kernel_real, np.float32),
                                  np.asarray(recurrent_kernel_imag, np.float32),
                                  np.asarray(bias_real, np.float32),
                                  np.asarray(bias_imag, np.float32))
    arg_map = {
        "x": x_dev,
        "h0": r.put(np.ascontiguousarray(np.asarray(h0, np.float32))),
        "c0": r.put(np.ascontiguousarray(np.asarray(c0, np.float32))),
        "wb": r.put_const("wb", _tile_cores(Wb)),
        "rb": r.put_const("rb", _tile_cores(Rb)),
        "bias": r.put_const("bias", _tile_cores(bias)),
        "id16": r.put_const("id16", _tile_cores(np.eye(128, dtype=np.float16))),
        "id32": r.put_const("id32", _tile_cores(np.eye(128, dtype=np.float32))),
    }
    (out_dev,) = r.run(arg_map)
    return _gather_f32(out_dev)


def _gather_f32(out_dev):
    """Fetch per-device output shards concurrently, widening fp16->fp32
    directly into the result (cast overlaps the next shard's transfer)."""
    import concurrent.futures as cf
    shards = list(out_dev.addressable_shards)
    shape = out_dev.shape
    out = np.empty(shape, np.float32)
    def fetch(s):
        idx = s.index[0]
        out[idx] = np.asarray(s.data)      # device_get + f16->f32 cast
        return (idx.stop or shape[0]) - (idx.start or 0)
    with cf.ThreadPoolExecutor(min(4, len(shards))) as ex:
        n = sum(ex.map(fetch, shards))
    assert n == shape[0], f"gather covered {n}/{shape[0]} rows"
    return out


if __name__ == "__main__":
    nc = _get_program()
    print("program built OK")


# revision 7
# speedup vs baseline: 16.1997x; 1.1217x over previous
"""Trainium2 Bass kernel for a complex-valued LSTM (nn_ComplexLSTMCell).

Math (per time step, complex arithmetic with real/imag stored split):
    z  = W x_t + R h_{t-1} + b          (complex affine, 4 gates x U units)
    i, f, o = sigmoid(z0, z1, z3);  g = tanh(z2)   (component-wise on re/im)
    c_t = f*c + i*g                      (complex elementwise products)
    h_t = o * tanh(c_t)                  (tanh applied component-wise to c_t)

Strategy: data-parallel across 8 NeuronCores (32 batch rows each).
Per core everything runs in a "z-transposed" layout [units(128 partitions),
batch(free)] so gate elementwise uses all 128 lanes:
  - x-projection zx = W x + b for a whole 64-step block is done with big
    matmuls (PE) and kept SBUF-resident in fp16.
  - per step: z = zx_t (injected into PSUM via identity-matmul) + 16
    accumulating [128,128]x[128,32] matmuls for R h.
  - gates on ScalarE (sigmoid/tanh, one table set), complex c/h updates
    as wide VectorE ops using strided APs.
  - h_t pairs are transposed back to batch-major via PE transpose and
    DMA'd out every 8 steps.

Host<->device path: x is shipped fp16 (the kernel rounded it to fp16 for
the matmuls anyway) and the output is produced fp16 on device and widened
to fp32 on the host; the PJRT executable, device-resident weights, and the
output-donation placeholder buffer are all built once and cached so warm
calls do no retracing/recompiling and transfer only x (in) and out (back).
"""
import os
import numpy as np

B, T, DIN, U = 256, 512, 64, 128
NCORES = 8
BL = B // NCORES          # 32 batch rows per core
TBLK = 64                 # steps per zx block
F2 = 2 * DIN              # 128: complex input features (re|im)
G8 = 8                    # gate chunks: f_r f_i i_r i_i o_r o_i g_r g_i

# gate index in reference weights: 0=i 1=f 2=g(tanh) 3=o
CHUNKS = [(1, 'r'), (1, 'i'), (0, 'r'), (0, 'i'), (3, 'r'), (3, 'i'), (2, 'r'), (2, 'i')]

_CACHE = {}


def _build_weights(kernel_real, kernel_imag, rec_real, rec_imag, bias_real, bias_imag):
    Wb = np.zeros((G8, F2, U), np.float32)       # (chunk, K=feat, M=units)
    Rb = np.zeros((2, G8, U, U), np.float32)     # (kchunk, chunk, K, M)
    bias = np.zeros((U, G8), np.float32)         # (unit, chunk)
    for c, (g, part) in enumerate(CHUNKS):
        cols = slice(g * U, (g + 1) * U)
        if part == 'r':
            Wb[c] = np.concatenate([kernel_real[:, cols], -kernel_imag[:, cols]], axis=0)
            Rb[0, c] = rec_real[:, cols]
            Rb[1, c] = -rec_imag[:, cols]
            bias[:, c] = bias_real[cols]
        else:
            Wb[c] = np.concatenate([kernel_imag[:, cols], kernel_real[:, cols]], axis=0)
            Rb[0, c] = rec_imag[:, cols]
            Rb[1, c] = rec_real[:, cols]
            bias[:, c] = bias_imag[cols]
    return Wb.astype(np.float16), Rb.astype(np.float16), bias


def _cap(tile_ap, col_offset, nest):
    """Column-strided AP: same tensor/partition dim, custom free-dim nest.

    nest: list of [step, count] in elements of the tile's free dim.
    """
    import concourse.bass as bass
    base = tile_ap[:, col_offset:col_offset + 1]
    return bass.AP(tensor=base.tensor, offset=base.offset,
                   ap=[list(base.ap[0])] + [list(p) for p in nest])


def _build_program(t_total=T, tblk=TBLK):
    import concourse.bacc as bacc
    import concourse.tile as tile
    from concourse import mybir
    from contextlib import ExitStack

    f16 = mybir.dt.float16
    f32 = mybir.dt.float32
    nblk = t_total // tblk
    Sig = mybir.ActivationFunctionType.Sigmoid
    Tanh = mybir.ActivationFunctionType.Tanh
    Copy = mybir.ActivationFunctionType.Copy
    Ident = mybir.ActivationFunctionType.Identity

    nc = bacc.Bacc("TRN2", target_bir_lowering=False, debug=False)

    x_d = nc.dram_tensor("x", [BL, t_total, F2], f16, kind="ExternalInput").ap()
    h0_d = nc.dram_tensor("h0", [BL, 2 * U], f32, kind="ExternalInput").ap()
    c0_d = nc.dram_tensor("c0", [BL, 2 * U], f32, kind="ExternalInput").ap()
    wb_d = nc.dram_tensor("wb", [G8, F2, U], f16, kind="ExternalInput").ap()
    rb_d = nc.dram_tensor("rb", [2, G8, U, U], f16, kind="ExternalInput").ap()
    bias_d = nc.dram_tensor("bias", [U, G8], f32, kind="ExternalInput").ap()
    id16_d = nc.dram_tensor("id16", [128, 128], f16, kind="ExternalInput").ap()
    id32_d = nc.dram_tensor("id32", [128, 128], f32, kind="ExternalInput").ap()
    out_d = nc.dram_tensor("out", [BL, t_total, 2 * U], f16, kind="ExternalOutput").ap()

    with tile.TileContext(nc) as tc, ExitStack() as ctx:
        consts = ctx.enter_context(tc.tile_pool(name="consts", bufs=1))
        state = ctx.enter_context(tc.tile_pool(name="state", bufs=1))
        xnatp = ctx.enter_context(tc.tile_pool(name="xnat", bufs=2))
        xtp = ctx.enter_context(tc.tile_pool(name="xTp", bufs=2))
        stagep = ctx.enter_context(tc.tile_pool(name="stagep", bufs=2))
        zsig_pool = ctx.enter_context(tc.tile_pool(name="zsig", bufs=2, space="PSUM"))
        zg_pool = ctx.enter_context(tc.tile_pool(name="zgp", bufs=2, space="PSUM"))
        htp_pool = ctx.enter_context(tc.tile_pool(name="htp", bufs=1, space="PSUM"))
        xps_pool = ctx.enter_context(tc.tile_pool(name="xps", bufs=2, space="PSUM"))

        # ---- constants ----
        W_sb = consts.tile([128, G8, U], f16)
        R_sb = consts.tile([128, 2, G8, U], f16)
        bias_sb = consts.tile([128, G8], f32)
        id16 = consts.tile([128, 128], f16)
        id32 = consts.tile([128, 128], f32)
        nc.sync.dma_start(out=W_sb, in_=wb_d.rearrange("c K m -> K c m"))
        nc.sync.dma_start(out=R_sb, in_=rb_d.rearrange("k c K m -> K k c m"))
        nc.sync.dma_start(out=bias_sb, in_=bias_d)
        nc.sync.dma_start(out=id16, in_=id16_d)
        nc.sync.dma_start(out=id32, in_=id32_d)

        # ---- state tiles ----
        CG = state.tile([128, 128], f16)      # [cr|ci|g_r|g_i]
        Hpair = state.tile([128, 128], f16)   # [hr_e|hi_e|hr_o|hi_o]
        A = state.tile([128, 6 * BL], f16)    # sigmoid outs [f_r f_i i_r i_i o_r o_i]
        Mt = state.tile([128, 256], f16)
        Sst = state.tile([128, 128], f16)
        TC = state.tile([128, 64], f16)
        zx_buf = state.tile([128, 2, G8, tblk * BL], f16)

        # ---- initial state: transpose h0/c0 into [unit, batch] layout ----
        hc_sb = state.tile([BL, 2 * (2 * U)], f32)
        nc.sync.dma_start(out=hc_sb[:, 0:2 * U], in_=h0_d)
        nc.sync.dma_start(out=hc_sb[:, 2 * U:], in_=c0_d)
        init_ps = htp_pool.tile([128, 128], f32, name="init_ps", tag="htp")
        for j in range(4):  # hr hi cr ci
            nc.tensor.transpose(init_ps[:, j * 32:(j + 1) * 32],
                                hc_sb[:, j * U:(j + 1) * U], id32[:BL, :BL])
        # h0 -> odd-parity slot (step 0 reads rpar=1), c0 -> CG[:, 0:64]
        nc.scalar.activation(Hpair[:, 64:128], init_ps[:, 0:64], Copy)
        nc.scalar.activation(CG[:, 0:64], init_ps[:, 64:128], Copy)

        # ---- x-phase emitters ----
        def emit_xphase_dma(blk):
            # x_nat rows = (t%4, b), tiles along t//4: 4 strided DMAs
            x_nat = xnatp.tile([128, tblk // 4, F2], f16, name="x_nat", tag="x_nat")
            t0 = blk * tblk
            for tp in range(4):
                nc.sync.dma_start(
                    out=x_nat[tp * BL:(tp + 1) * BL, :, :],
                    in_=x_d[:, t0 + tp:t0 + tblk:4, :])
            xT = xtp.tile([128, tblk // 4, F2], f16, name="xT", tag="xT")
            return x_nat, xT

        def emit_xphase_transpose(x_nat, xT, i):
            # transpose 4 [128,128] chunks into one PSUM bank, cast to fp16
            xt_ps = xps_pool.tile([128, 512], f16, name="xt_ps", tag="xps")
            for j in range(4):
                nc.tensor.transpose(xt_ps[:, j * 128:(j + 1) * 128],
                                    x_nat[:, 4 * i + j, :], id16)
            nc.vector.tensor_copy(xT[:, 4 * i:4 * i + 4, :], xt_ps)

        def emit_xphase_mm(xT, blk, c, j):
            # zx[c, j*512:(j+1)*512] for block blk, cast + bias to fp16 SBUF
            bb = blk % 2
            zx_ps = xps_pool.tile([128, 512], f32, name="zx_ps", tag="xps")
            nc.tensor.matmul(zx_ps, lhsT=W_sb[:, c, :], rhs=xT[:, 4 * j:4 * j + 4, :],
                             start=True, stop=True)
            dst = zx_buf[:, bb, c, j * 512:(j + 1) * 512]
            if (c + j) % 2 == 0:
                nc.scalar.activation(dst, zx_ps, Ident, bias=bias_sb[:, c:c + 1])
            else:
                nc.vector.tensor_scalar_add(dst, zx_ps, bias_sb[:, c:c + 1])

        # ---- one recurrence step ----
        def emit_step(t):
            blk = t // tblk
            tl = t % tblk
            bb = blk % 2
            par = t % 2
            rpar = (t + 1) % 2  # parity slot holding h_{t-1}

            zs = zsig_pool.tile([128, 6 * BL], f32, name="zs", tag="zs")
            zg = zg_pool.tile([128, 2 * BL], f32, name="zg", tag="zg")

            # --- PE: z = zx_t + R h ---
            zx_s = zx_buf[:, bb, 0:6, tl * BL:(tl + 1) * BL]
            zx_g = zx_buf[:, bb, 6:8, tl * BL:(tl + 1) * BL]
            nc.tensor.matmul(zs, lhsT=id16, rhs=zx_s, start=True, stop=False)
            for k in range(2):
                hk = Hpair[:, rpar * 64 + k * BL: rpar * 64 + (k + 1) * BL]
                for c in range(6):
                    nc.tensor.matmul(zs[:, c * BL:(c + 1) * BL], lhsT=R_sb[:, k, c, :],
                                     rhs=hk, start=False, stop=(k == 1 and c == 5))
            nc.tensor.matmul(zg, lhsT=id16, rhs=zx_g, start=True, stop=False)
            for k in range(2):
                hk = Hpair[:, rpar * 64 + k * BL: rpar * 64 + (k + 1) * BL]
                for c in range(6, 8):
                    nc.tensor.matmul(zg[:, (c - 6) * BL:(c - 5) * BL], lhsT=R_sb[:, k, c, :],
                                     rhs=hk, start=False, stop=(k == 1 and c == 7))

            # --- ACT: gates ---
            nc.scalar.activation(A, zs, Sig)
            nc.scalar.activation(CG[:, 64:128], zg, Tanh)

            # --- DVE: complex c update ---
            # M1 = [f_r f_i i_r i_i] * [cr ci g_r g_i]
            nc.vector.tensor_mul(Mt[:, 0:128], A[:, 0:128], CG[:, 0:128])
            # M2 = [f_r f_i i_r i_i] * [ci cr g_i g_r]
            nc.vector.tensor_mul(Mt[:, 128:256], A[:, 0:128],
                                 _cap(CG, 32, [[64, 2], [-32, 2], [1, 32]]))
            # S1 = [f_r*cr - f_i*ci | i_r*g_r - i_i*g_i]
            nc.vector.tensor_sub(Sst[:, 0:64],
                                 _cap(Mt, 0, [[64, 2], [1, 32]]),
                                 _cap(Mt, 32, [[64, 2], [1, 32]]))
            # S2 = f_r*ci + f_i*cr ; S3 = i_r*g_i - i_i*g_r
            nc.vector.tensor_add(Sst[:, 64:96], Mt[:, 128:160], Mt[:, 160:192])
            nc.vector.tensor_sub(Sst[:, 96:128], Mt[:, 192:224], Mt[:, 224:256])
            # C = [S1a+S1b | S2+S3]
            nc.vector.tensor_add(CG[:, 0:64],
                                 _cap(Sst, 0, [[64, 2], [1, 32]]),
                                 _cap(Sst, 32, [[64, 2], [1, 32]]))

            # --- ACT: tanh of c ---
            nc.scalar.activation(TC, CG[:, 0:64], Tanh)

            # --- DVE: h = o * tanh_c (complex) ---
            nc.vector.tensor_mul(Mt[:, 0:64], A[:, 128:192], TC)
            nc.vector.tensor_mul(Mt[:, 64:128], A[:, 128:192],
                                 _cap(TC, 32, [[-32, 2], [1, 32]]))
            # hr = o_r*tcr - o_i*tci ; hi = o_r*tci - o_i*tcr  (both minus -> 1 op)
            nc.vector.tensor_sub(Hpair[:, par * 64: par * 64 + 64],
                                 _cap(Mt, 0, [[64, 2], [1, 32]]),
                                 _cap(Mt, 32, [[64, 2], [1, 32]]))

        # ---- output staging ----
        def emit_hout(t, stage_tile):
            # after odd step t: transpose (t-1,t) h pair into stage col (t//2)%4
            jp = (t // 2) % 4
            tp_ps = htp_pool.tile([128, 128], f16, name="tp_ps", tag="htp")
            nc.tensor.transpose(tp_ps, Hpair, id16)
            nc.scalar.activation(stage_tile[:, jp, :], tp_ps, Copy)

        def emit_hout_dma(t, stage_tile):
            # after step t (t%8==7): DMA stage -> out[t-7 .. t].
            # stage partition = (tpar, half, b); 4 DMAs, one per (tpar, half).
            t0 = t - 7
            for tpar in range(2):
                for h in range(2):
                    p0 = tpar * 64 + h * 32
                    nc.sync.dma_start(
                        out=out_d[:, t0 + tpar:t0 + 8:2, h * U:(h + 1) * U],
                        in_=stage_tile[p0:p0 + 32, :, :])

        # ---- prologue: x-phase for block 0 ----
        x_nat, xT = emit_xphase_dma(0)
        for i in range(tblk // 16):
            emit_xphase_transpose(x_nat, xT, i)
        for c in range(8):
            for j in range(tblk // 16):
                emit_xphase_mm(xT, 0, c, j)

        # ---- main loop (fully unrolled) ----
        stage_tile = None
        for blk in range(nblk):
            nxt = blk + 1
            xph = []
            if nxt < nblk:
                x_nat, xT = emit_xphase_dma(nxt)
                xph += [('t', i) for i in range(tblk // 16)]
                xph += [('m', c, j) for c in range(8) for j in range(tblk // 16)]
            for tl in range(tblk):
                t = blk * tblk + tl
                if t % 8 == 0:
                    stage_tile = stagep.tile([128, 4, 128], f16,
                                             name="stage", tag="stage")
                emit_step(t)
                if t % 2 == 1:
                    emit_hout(t, stage_tile)
                if t % 8 == 7:
                    emit_hout_dma(t, stage_tile)
                # spread next-block x-phase work across this block's steps
                while xph and len(xph) > (tblk - 1 - tl):
                    op = xph.pop(0)
                    if op[0] == 't':
                        emit_xphase_transpose(x_nat, xT, op[1])
                    else:
                        emit_xphase_mm(xT, nxt, op[1], op[2])

    nc.compile()
    return nc


def _get_program(t_total=T, tblk=TBLK):
    key = (t_total, tblk)
    if key not in _CACHE:
        _CACHE[key] = _build_program(t_total, tblk)
    return _CACHE[key]


# ---------------------------------------------------------------------------
# Cached PJRT execution path.
#
# bass_utils.run_bass_kernel_spmd -> bass2jax.run_bass_via_pjrt builds a fresh
# jax.jit closure on every call, so warm calls re-trace, re-serialize the BIR
# into the HLO, and re-run XLA/NEFF compilation; it also ships host-built zero
# output buffers (full fp32 output size) over the relay each call. This class
# replicates its lowering exactly but builds the jitted executable once, keeps
# weights/constants device-resident, and keeps a device-side placeholder for
# the output operand (the kernel writes every output element, so the
# placeholder contents are never observed and it can be reused, undonated).
# ---------------------------------------------------------------------------
class _Runner:
    def __init__(self, nc, n_cores):
        import jax
        import jax.numpy as jnp
        from jax.sharding import Mesh, NamedSharding, PartitionSpec
        from jax.experimental.shard_map import shard_map
        from concourse import bass2jax, mybir

        bass2jax.install_neuronx_cc_hook()
        assert nc.dbg_addr is None, "build the program with debug=False"

        partition_name = (nc.partition_id_tensor.name
                          if nc.partition_id_tensor else None)
        in_names, out_names, out_avals = [], [], []
        for alloc in nc.m.functions[0].allocations:
            if not isinstance(alloc, mybir.MemoryLocationSet):
                continue
            name = alloc.memorylocations[0].name
            if alloc.kind == "ExternalInput":
                if name != partition_name:
                    in_names.append(name)
            elif alloc.kind == "ExternalOutput":
                shape = tuple(alloc.tensor_shape)
                dtype = mybir.dt.np(alloc.dtype)
                out_names.append(name)
                out_avals.append(jax.core.ShapedArray(shape, dtype))
        self.param_names = list(in_names)
        in_names = in_names + out_names
        if partition_name is not None:
            in_names.append(partition_name)

        def _body(*args):
            operands = list(args)
            if partition_name is not None:
                operands.append(bass2jax.partition_id_tensor())
            outs = bass2jax._bass_exec_p.bind(
                *operands,
                out_avals=tuple(out_avals),
                in_names=tuple(in_names),
                out_names=tuple(out_names),
                lowering_input_output_aliases=(),
                sim_require_finite=True,
                sim_require_nnan=True,
                nc=nc,
            )
            return tuple(outs)

        devices = [d for d in jax.devices() if d.platform != "cpu"]
        if len(devices) < n_cores:   # neuron cores on a non-default platform
            for plat in ("axon", "neuron"):
                try:
                    devices = jax.devices(plat)
                    break
                except RuntimeError:
                    continue
        devices = devices[:n_cores]
        assert len(devices) == n_cores, f"need {n_cores} neuron cores"
        mesh = Mesh(np.asarray(devices), ("core",))
        self.sharding = NamedSharding(mesh, PartitionSpec("core"))
        n_args = len(self.param_names) + len(out_names)
        self.fn = jax.jit(
            shard_map(_body, mesh=mesh,
                      in_specs=(PartitionSpec("core"),) * n_args,
                      out_specs=(PartitionSpec("core"),) * len(out_names),
                      check_rep=False),
            keep_unused=True,
        )
        # device-side placeholder for each output operand (never read back)
        self._outbuf_fn = jax.jit(
            lambda: tuple(jnp.zeros((n_cores * a.shape[0],) + a.shape[1:], a.dtype)
                          for a in out_avals),
            out_shardings=(self.sharding,) * len(out_avals),
        )
        self._outbufs = None
        self._jax = jax
        self._const_host = {}    # name -> host array last uploaded
        self._const_dev = {}     # name -> device array

    def put(self, arr):
        """Async upload of a full (n_cores*d0, ...) host array, sharded on axis 0."""
        return self._jax.device_put(arr, self.sharding)

    def put_const(self, name, arr):
        """Device-cached upload: re-uploads only if contents changed."""
        prev = self._const_host.get(name)
        if prev is not None and prev.shape == arr.shape and np.array_equal(prev, arr):
            return self._const_dev[name]
        dev = self.put(arr)
        self._const_host[name] = arr
        self._const_dev[name] = dev
        return dev

    def run(self, arg_map):
        if self._outbufs is None:
            self._outbufs = self._outbuf_fn()
        args = [arg_map[n] for n in self.param_names]
        return self.fn(*args, *self._outbufs)


_RUNNERS = {}


def _get_runner(t_total=T, tblk=TBLK):
    key = (t_total, tblk)
    if key not in _RUNNERS:
        _RUNNERS[key] = _Runner(_get_program(t_total, tblk), NCORES)
    return _RUNNERS[key]


def _tile_cores(a):
    """Replicate a per-core array n_cores times along axis 0."""
    return np.broadcast_to(a, (NCORES,) + a.shape).reshape(
        (NCORES * a.shape[0],) + a.shape[1:])


def kernel(x, h0, c0, kernel_real, kernel_imag,
           recurrent_kernel_real, recurrent_kernel_imag,
           bias_real, bias_imag, _t_total=T, _tblk=TBLK):
    x = np.asarray(x)
    r = _get_runner(_t_total, _tblk)

    # start the big x upload first so it overlaps the host-side prep below
    x16 = x.astype(np.float16) if x.dtype != np.float16 else x
    x_dev = r.put(np.ascontiguousarray(x16))

    Wb, Rb, bias = _build_weights(np.asarray(kernel_real, np.float32),
                                  np.asarray(kernel_imag, np.float32),
                                  np.asarray(recurrent_kernel_real, np.float32),
                                  np.asarray(recurrent_kernel_imag, np.float32),
                                  np.asarray(bias_real, np.float32),
                                  np.asarray(bias_imag, np.float32))
    arg_map = {
        "x": x_dev,
        "h0": r.put(np.ascontiguousarray(np.asarray(h0, np.float32))),
        "c0": r.put(np.ascontiguousarray(np.asarray(c0, np.float32))),
        "wb": r.put_const("wb", _tile_cores(Wb)),
        "rb": r.put_const("rb", _tile_cores(Rb)),
        "bias": r.put_const("bias", _tile_cores(bias)),
        "id16": r.put_const("id16", _tile_cores(np.eye(128, dtype=np.float16))),
        "id32": r.put_const("id32", _tile_cores(np.eye(128, dtype=np.float32))),
    }
    (out_dev,) = r.run(arg_map)
    return _gather_f32(out_dev)


def _gather_f32(out_dev):
    """Fetch per-device output shards concurrently, widening fp16->fp32
    directly into the result (cast overlaps the next shard's transfer)."""
    import concurrent.futures as cf
    shards = list(out_dev.addressable_shards)
    shape = out_dev.shape
    out = np.empty(shape, np.float32)
    def fetch(s):
        idx = s.index[0]
        out[idx] = np.asarray(s.data)      # device_get + f16->f32 cast
        return (idx.stop or shape[0]) - (idx.start or 0)
    with cf.ThreadPoolExecutor(min(4, len(shards))) as ex:
        n = sum(ex.map(fetch, shards))
    assert n == shape[0], f"gather covered {n}/{shape[0]} rows"
    return out


if __name__ == "__main__":
    nc = _get_program()
    print("program built OK")


# revision 9
# speedup vs baseline: 17.9679x; 1.1092x over previous
"""Trainium2 Bass kernel for a complex-valued LSTM (nn_ComplexLSTMCell).

Math (per time step, complex arithmetic with real/imag stored split):
    z  = W x_t + R h_{t-1} + b          (complex affine, 4 gates x U units)
    i, f, o = sigmoid(z0, z1, z3);  g = tanh(z2)   (component-wise on re/im)
    c_t = f*c + i*g                      (complex elementwise products)
    h_t = o * tanh(c_t)                  (tanh applied component-wise to c_t)

Strategy: data-parallel across 8 NeuronCores (32 batch rows each).
Per core everything runs in a "z-transposed" layout [units(128 partitions),
batch(free)] so gate elementwise uses all 128 lanes:
  - x-projection zx = W x + b for a whole 64-step block is done with big
    matmuls (PE) and kept SBUF-resident in fp16.
  - per step: z = zx_t (injected into PSUM via identity-matmul) + 16
    accumulating [128,128]x[128,32] matmuls for R h.
  - gates on ScalarE (sigmoid/tanh, one table set), complex c/h updates
    as wide VectorE ops using strided APs.
  - h_t pairs are transposed back to batch-major via PE transpose and
    DMA'd out every 8 steps.

Host<->device path: x is shipped fp16 (the kernel rounded it to fp16 for
the matmuls anyway) and the output is produced fp16 on device and widened
to fp32 on the host; the PJRT executable, device-resident weights, and the
output-donation placeholder buffer are all built once and cached so warm
calls do no retracing/recompiling and transfer only x (in) and out (back).
"""
import os
import numpy as np

B, T, DIN, U = 256, 512, 64, 128
NCORES = 8
BL = B // NCORES          # 32 batch rows per core
TBLK = 64                 # steps per zx block
F2 = 2 * DIN              # 128: complex input features (re|im)
G8 = 8                    # gate chunks: f_r f_i i_r i_i o_r o_i g_r g_i

# gate index in reference weights: 0=i 1=f 2=g(tanh) 3=o
CHUNKS = [(1, 'r'), (1, 'i'), (0, 'r'), (0, 'i'), (3, 'r'), (3, 'i'), (2, 'r'), (2, 'i')]

_CACHE = {}


def _build_weights(kernel_real, kernel_imag, rec_real, rec_imag, bias_real, bias_imag):
    Wb = np.zeros((G8, F2, U), np.float32)       # (chunk, K=feat, M=units)
    Rb = np.zeros((2, G8, U, U), np.float32)     # (kchunk, chunk, K, M)
    bias = np.zeros((U, G8), np.float32)         # (unit, chunk)
    for c, (g, part) in enumerate(CHUNKS):
        cols = slice(g * U, (g + 1) * U)
        if part == 'r':
            Wb[c] = np.concatenate([kernel_real[:, cols], -kernel_imag[:, cols]], axis=0)
            Rb[0, c] = rec_real[:, cols]
            Rb[1, c] = -rec_imag[:, cols]
            bias[:, c] = bias_real[cols]
        else:
            Wb[c] = np.concatenate([kernel_imag[:, cols], kernel_real[:, cols]], axis=0)
            Rb[0, c] = rec_imag[:, cols]
            Rb[1, c] = rec_real[:, cols]
            bias[:, c] = bias_imag[cols]
    return Wb.astype(np.float16), Rb.astype(np.float16), bias


def _cap(tile_ap, col_offset, nest):
    """Column-strided AP: same tensor/partition dim, custom free-dim nest.

    nest: list of [step, count] in elements of the tile's free dim.
    """
    import concourse.bass as bass
    base = tile_ap[:, col_offset:col_offset + 1]
    return bass.AP(tensor=base.tensor, offset=base.offset,
                   ap=[list(base.ap[0])] + [list(p) for p in nest])


def _build_program(t_total=T, tblk=TBLK):
    import concourse.bacc as bacc
    import concourse.tile as tile
    from concourse import mybir
    from contextlib import ExitStack

    f16 = mybir.dt.float16
    f32 = mybir.dt.float32
    nblk = t_total // tblk
    Sig = mybir.ActivationFunctionType.Sigmoid
    Tanh = mybir.ActivationFunctionType.Tanh
    Copy = mybir.ActivationFunctionType.Copy
    Ident = mybir.ActivationFunctionType.Identity

    nc = bacc.Bacc("TRN2", target_bir_lowering=False, debug=False)

    x_d = nc.dram_tensor("x", [BL, t_total, F2], f16, kind="ExternalInput").ap()
    h0_d = nc.dram_tensor("h0", [BL, 2 * U], f32, kind="ExternalInput").ap()
    c0_d = nc.dram_tensor("c0", [BL, 2 * U], f32, kind="ExternalInput").ap()
    wb_d = nc.dram_tensor("wb", [G8, F2, U], f16, kind="ExternalInput").ap()
    rb_d = nc.dram_tensor("rb", [2, G8, U, U], f16, kind="ExternalInput").ap()
    bias_d = nc.dram_tensor("bias", [U, G8], f32, kind="ExternalInput").ap()
    id16_d = nc.dram_tensor("id16", [128, 128], f16, kind="ExternalInput").ap()
    id32_d = nc.dram_tensor("id32", [128, 128], f32, kind="ExternalInput").ap()
    out_d = nc.dram_tensor("out", [BL, t_total, 2 * U], f16, kind="ExternalOutput").ap()

    with tile.TileContext(nc) as tc, ExitStack() as ctx:
        consts = ctx.enter_context(tc.tile_pool(name="consts", bufs=1))
        state = ctx.enter_context(tc.tile_pool(name="state", bufs=1))
        xnatp = ctx.enter_context(tc.tile_pool(name="xnat", bufs=2))
        xtp = ctx.enter_context(tc.tile_pool(name="xTp", bufs=2))
        stagep = ctx.enter_context(tc.tile_pool(name="stagep", bufs=2))
        zsig_pool = ctx.enter_context(tc.tile_pool(name="zsig", bufs=2, space="PSUM"))
        zg_pool = ctx.enter_context(tc.tile_pool(name="zgp", bufs=2, space="PSUM"))
        htp_pool = ctx.enter_context(tc.tile_pool(name="htp", bufs=1, space="PSUM"))
        xps_pool = ctx.enter_context(tc.tile_pool(name="xps", bufs=2, space="PSUM"))

        # ---- constants ----
        W_sb = consts.tile([128, G8, U], f16)
        R_sb = consts.tile([128, 2, G8, U], f16)
        bias_sb = consts.tile([128, G8], f32)
        id16 = consts.tile([128, 128], f16)
        id32 = consts.tile([128, 128], f32)
        nc.sync.dma_start(out=W_sb, in_=wb_d.rearrange("c K m -> K c m"))
        nc.sync.dma_start(out=R_sb, in_=rb_d.rearrange("k c K m -> K k c m"))
        nc.sync.dma_start(out=bias_sb, in_=bias_d)
        nc.sync.dma_start(out=id16, in_=id16_d)
        nc.sync.dma_start(out=id32, in_=id32_d)

        # ---- state tiles ----
        CG = state.tile([128, 128], f16)      # [cr|ci|g_r|g_i]
        Hpair = state.tile([128, 128], f16)   # [hr_e|hi_e|hr_o|hi_o]
        A = state.tile([128, 6 * BL], f16)    # sigmoid outs [f_r f_i i_r i_i o_r o_i]
        Mt = state.tile([128, 256], f16)
        Sst = state.tile([128, 128], f16)
        TC = state.tile([128, 64], f16)
        zx_buf = state.tile([128, 2, G8, tblk * BL], f16)

        # ---- initial state: transpose h0/c0 into [unit, batch] layout ----
        hc_sb = state.tile([BL, 2 * (2 * U)], f32)
        nc.sync.dma_start(out=hc_sb[:, 0:2 * U], in_=h0_d)
        nc.sync.dma_start(out=hc_sb[:, 2 * U:], in_=c0_d)
        init_ps = htp_pool.tile([128, 128], f32, name="init_ps", tag="htp")
        for j in range(4):  # hr hi cr ci
            nc.tensor.transpose(init_ps[:, j * 32:(j + 1) * 32],
                                hc_sb[:, j * U:(j + 1) * U], id32[:BL, :BL])
        # h0 -> odd-parity slot (step 0 reads rpar=1), c0 -> CG[:, 0:64]
        nc.scalar.activation(Hpair[:, 64:128], init_ps[:, 0:64], Copy)
        nc.scalar.activation(CG[:, 0:64], init_ps[:, 64:128], Copy)

        # ---- x-phase emitters ----
        def emit_xphase_dma(blk):
            # x_nat rows = (t%4, b), tiles along t//4: 4 strided DMAs
            x_nat = xnatp.tile([128, tblk // 4, F2], f16, name="x_nat", tag="x_nat")
            t0 = blk * tblk
            for tp in range(4):
                nc.sync.dma_start(
                    out=x_nat[tp * BL:(tp + 1) * BL, :, :],
                    in_=x_d[:, t0 + tp:t0 + tblk:4, :])
            xT = xtp.tile([128, tblk // 4, F2], f16, name="xT", tag="xT")
            return x_nat, xT

        def emit_xphase_transpose(x_nat, xT, i):
            # transpose 4 [128,128] chunks into one PSUM bank, cast to fp16
            xt_ps = xps_pool.tile([128, 512], f16, name="xt_ps", tag="xps")
            for j in range(4):
                nc.tensor.transpose(xt_ps[:, j * 128:(j + 1) * 128],
                                    x_nat[:, 4 * i + j, :], id16)
            nc.vector.tensor_copy(xT[:, 4 * i:4 * i + 4, :], xt_ps)

        def emit_xphase_mm(xT, blk, c, j):
            # zx[c, j*512:(j+1)*512] for block blk, cast + bias to fp16 SBUF
            bb = blk % 2
            zx_ps = xps_pool.tile([128, 512], f32, name="zx_ps", tag="xps")
            nc.tensor.matmul(zx_ps, lhsT=W_sb[:, c, :], rhs=xT[:, 4 * j:4 * j + 4, :],
                             start=True, stop=True)
            dst = zx_buf[:, bb, c, j * 512:(j + 1) * 512]
            if (c + j) % 2 == 0:
                nc.scalar.activation(dst, zx_ps, Ident, bias=bias_sb[:, c:c + 1])
            else:
                nc.vector.tensor_scalar_add(dst, zx_ps, bias_sb[:, c:c + 1])

        # ---- one recurrence step ----
        def emit_step(t):
            blk = t // tblk
            tl = t % tblk
            bb = blk % 2
            par = t % 2
            rpar = (t + 1) % 2  # parity slot holding h_{t-1}

            zs = zsig_pool.tile([128, 6 * BL], f32, name="zs", tag="zs")
            zg = zg_pool.tile([128, 2 * BL], f32, name="zg", tag="zg")

            # --- PE: z = zx_t + R h ---
            zx_s = zx_buf[:, bb, 0:6, tl * BL:(tl + 1) * BL]
            zx_g = zx_buf[:, bb, 6:8, tl * BL:(tl + 1) * BL]
            nc.tensor.matmul(zs, lhsT=id16, rhs=zx_s, start=True, stop=False)
            for k in range(2):
                hk = Hpair[:, rpar * 64 + k * BL: rpar * 64 + (k + 1) * BL]
                for c in range(6):
                    nc.tensor.matmul(zs[:, c * BL:(c + 1) * BL], lhsT=R_sb[:, k, c, :],
                                     rhs=hk, start=False, stop=(k == 1 and c == 5))
            nc.tensor.matmul(zg, lhsT=id16, rhs=zx_g, start=True, stop=False)
            for k in range(2):
                hk = Hpair[:, rpar * 64 + k * BL: rpar * 64 + (k + 1) * BL]
                for c in range(6, 8):
                    nc.tensor.matmul(zg[:, (c - 6) * BL:(c - 5) * BL], lhsT=R_sb[:, k, c, :],
                                     rhs=hk, start=False, stop=(k == 1 and c == 7))

            # --- ACT: gates ---
            nc.scalar.activation(A, zs, Sig)
            nc.scalar.activation(CG[:, 64:128], zg, Tanh)

            # --- DVE: complex c update ---
            # M1 = [f_r f_i i_r i_i] * [cr ci g_r g_i]
            nc.vector.tensor_mul(Mt[:, 0:128], A[:, 0:128], CG[:, 0:128])
            # M2 = [f_r f_i i_r i_i] * [ci cr g_i g_r]
            nc.vector.tensor_mul(Mt[:, 128:256], A[:, 0:128],
                                 _cap(CG, 32, [[64, 2], [-32, 2], [1, 32]]))
            # S1 = [f_r*cr - f_i*ci | i_r*g_r - i_i*g_i]
            nc.vector.tensor_sub(Sst[:, 0:64],
                                 _cap(Mt, 0, [[64, 2], [1, 32]]),
                                 _cap(Mt, 32, [[64, 2], [1, 32]]))
            # S2 = f_r*ci + f_i*cr ; S3 = i_r*g_i - i_i*g_r
            nc.vector.tensor_add(Sst[:, 64:96], Mt[:, 128:160], Mt[:, 160:192])
            nc.vector.tensor_sub(Sst[:, 96:128], Mt[:, 192:224], Mt[:, 224:256])
            # C = [S1a+S1b | S2+S3]
            nc.vector.tensor_add(CG[:, 0:64],
                                 _cap(Sst, 0, [[64, 2], [1, 32]]),
                                 _cap(Sst, 32, [[64, 2], [1, 32]]))

            # --- ACT: tanh of c ---
            nc.scalar.activation(TC, CG[:, 0:64], Tanh)

            # --- DVE: h = o * tanh_c (complex) ---
            nc.vector.tensor_mul(Mt[:, 0:64], A[:, 128:192], TC)
            nc.vector.tensor_mul(Mt[:, 64:128], A[:, 128:192],
                                 _cap(TC, 32, [[-32, 2], [1, 32]]))
            # hr = o_r*tcr - o_i*tci ; hi = o_r*tci - o_i*tcr  (both minus -> 1 op)
            nc.vector.tensor_sub(Hpair[:, par * 64: par * 64 + 64],
                                 _cap(Mt, 0, [[64, 2], [1, 32]]),
                                 _cap(Mt, 32, [[64, 2], [1, 32]]))

        # ---- output staging ----
        def emit_hout(t, stage_tile):
            # after odd step t: transpose (t-1,t) h pair into stage col (t//2)%4
            jp = (t // 2) % 4
            tp_ps = htp_pool.tile([128, 128], f16, name="tp_ps", tag="htp")
            nc.tensor.transpose(tp_ps, Hpair, id16)
            nc.scalar.activation(stage_tile[:, jp, :], tp_ps, Copy)

        def emit_hout_dma(t, stage_tile):
            # after step t (t%8==7): DMA stage -> out[t-7 .. t].
            # stage partition = (tpar, half, b); 4 DMAs, one per (tpar, half).
            t0 = t - 7
            for tpar in range(2):
                for h in range(2):
                    p0 = tpar * 64 + h * 32
                    nc.sync.dma_start(
                        out=out_d[:, t0 + tpar:t0 + 8:2, h * U:(h + 1) * U],
                        in_=stage_tile[p0:p0 + 32, :, :])

        # ---- prologue: x-phase for block 0 ----
        x_nat, xT = emit_xphase_dma(0)
        for i in range(tblk // 16):
            emit_xphase_transpose(x_nat, xT, i)
        for c in range(8):
            for j in range(tblk // 16):
                emit_xphase_mm(xT, 0, c, j)

        # ---- main loop (fully unrolled) ----
        stage_tile = None
        for blk in range(nblk):
            nxt = blk + 1
            xph = []
            if nxt < nblk:
                x_nat, xT = emit_xphase_dma(nxt)
                xph += [('t', i) for i in range(tblk // 16)]
                xph += [('m', c, j) for c in range(8) for j in range(tblk // 16)]
            for tl in range(tblk):
                t = blk * tblk + tl
                if t % 8 == 0:
                    stage_tile = stagep.tile([128, 4, 128], f16,
                                             name="stage", tag="stage")
                emit_step(t)
                if t % 2 == 1:
                    emit_hout(t, stage_tile)
                if t % 8 == 7:
                    emit_hout_dma(t, stage_tile)
                # spread next-block x-phase work across this block's steps
                while xph and len(xph) > (tblk - 1 - tl):
                    op = xph.pop(0)
                    if op[0] == 't':
                        emit_xphase_transpose(x_nat, xT, op[1])
                    else:
                        emit_xphase_mm(xT, nxt, op[1], op[2])

    nc.compile()
    return nc


def _get_program(t_total=T, tblk=TBLK):
    key = (t_total, tblk)
    if key not in _CACHE:
        _CACHE[key] = _build_program(t_total, tblk)
    return _CACHE[key]


# ---------------------------------------------------------------------------
# Cached PJRT execution path.
#
# bass_utils.run_bass_kernel_spmd -> bass2jax.run_bass_via_pjrt builds a fresh
# jax.jit closure on every call, so warm calls re-trace, re-serialize the BIR
# into the HLO, and re-run XLA/NEFF compilation; it also ships host-built zero
# output buffers (full fp32 output size) over the relay each call. This class
# replicates its lowering exactly but builds the jitted executable once, keeps
# weights/constants device-resident, and keeps a device-side placeholder for
# the output operand (the kernel writes every output element, so the
# placeholder contents are never observed and it can be reused, undonated).
# ---------------------------------------------------------------------------
class _Runner:
    def __init__(self, nc, n_cores):
        import jax
        import jax.numpy as jnp
        from jax.sharding import Mesh, NamedSharding, PartitionSpec
        from jax.experimental.shard_map import shard_map
        from concourse import bass2jax, mybir

        bass2jax.install_neuronx_cc_hook()
        assert nc.dbg_addr is None, "build the program with debug=False"

        partition_name = (nc.partition_id_tensor.name
                          if nc.partition_id_tensor else None)
        in_names, out_names, out_avals = [], [], []
        for alloc in nc.m.functions[0].allocations:
            if not isinstance(alloc, mybir.MemoryLocationSet):
                continue
            name = alloc.memorylocations[0].name
            if alloc.kind == "ExternalInput":
                if name != partition_name:
                    in_names.append(name)
            elif alloc.kind == "ExternalOutput":
                shape = tuple(alloc.tensor_shape)
                dtype = mybir.dt.np(alloc.dtype)
                out_names.append(name)
                out_avals.append(jax.core.ShapedArray(shape, dtype))
        self.param_names = list(in_names)
        in_names = in_names + out_names
        if partition_name is not None:
            in_names.append(partition_name)

        def _body(*args):
            operands = list(args)
            if partition_name is not None:
                operands.append(bass2jax.partition_id_tensor())
            outs = bass2jax._bass_exec_p.bind(
                *operands,
                out_avals=tuple(out_avals),
                in_names=tuple(in_names),
                out_names=tuple(out_names),
                lowering_input_output_aliases=(),
                sim_require_finite=True,
                sim_require_nnan=True,
                nc=nc,
            )
            return tuple(outs)

        devices = [d for d in jax.devices() if d.platform != "cpu"]
        if len(devices) < n_cores:   # neuron cores on a non-default platform
            for plat in ("axon", "neuron"):
                try:
                    devices = jax.devices(plat)
                    break
                except RuntimeError:
                    continue
        devices = devices[:n_cores]
        assert len(devices) == n_cores, f"need {n_cores} neuron cores"
        mesh = Mesh(np.asarray(devices), ("core",))
        self.sharding = NamedSharding(mesh, PartitionSpec("core"))
        n_args = len(self.param_names) + len(out_names)
        self.fn = jax.jit(
            shard_map(_body, mesh=mesh,
                      in_specs=(PartitionSpec("core"),) * n_args,
                      out_specs=(PartitionSpec("core"),) * len(out_names),
                      check_rep=False),
            keep_unused=True,
        )
        # device-side placeholder for each output operand (never read back)
        self._outbuf_fn = jax.jit(
            lambda: tuple(jnp.zeros((n_cores * a.shape[0],) + a.shape[1:], a.dtype)
                          for a in out_avals),
            out_shardings=(self.sharding,) * len(out_avals),
        )
        self._outbufs = None
        self._jax = jax
        self._const_host = {}    # name -> host array last uploaded
        self._const_dev = {}     # name -> device array

    def put(self, arr):
        """Async upload of a full (n_cores*d0, ...) host array, sharded on axis 0."""
        return self._jax.device_put(arr, self.sharding)

    def put_x(self, x):
        """Chunked cast+upload of x: cast one core's slice, start its async
        transfer, then cast the next (overlaps host cast with relay upload)."""
        jax = self._jax
        devices = self.sharding.mesh.devices.reshape(-1)
        n = len(devices)
        d0 = x.shape[0] // n
        shards = []
        for i, dev in enumerate(devices):
            sl = np.ascontiguousarray(
                np.asarray(x[i * d0:(i + 1) * d0]).astype(np.float16, copy=False))
            shards.append(jax.device_put(sl, dev))
        return jax.make_array_from_single_device_arrays(
            x.shape, self.sharding, shards)

    def put_const(self, name, arr):
        """Device-cached upload: re-uploads only if contents changed."""
        prev = self._const_host.get(name)
        if prev is not None and prev.shape == arr.shape and np.array_equal(prev, arr):
            return self._const_dev[name]
        dev = self.put(arr)
        self._const_host[name] = arr
        self._const_dev[name] = dev
        return dev

    def run(self, arg_map):
        if self._outbufs is None:
            self._outbufs = self._outbuf_fn()
        args = [arg_map[n] for n in self.param_names]
        return self.fn(*args, *self._outbufs)


_RUNNERS = {}


def _get_runner(t_total=T, tblk=TBLK):
    key = (t_total, tblk)
    if key not in _RUNNERS:
        _RUNNERS[key] = _Runner(_get_program(t_total, tblk), NCORES)
    return _RUNNERS[key]


def _tile_cores(a):
    """Replicate a per-core array n_cores times along axis 0."""
    return np.broadcast_to(a, (NCORES,) + a.shape).reshape(
        (NCORES * a.shape[0],) + a.shape[1:])


def kernel(x, h0, c0, kernel_real, kernel_imag,
           recurrent_kernel_real, recurrent_kernel_imag,
           bias_real, bias_imag, _t_total=T, _tblk=TBLK):
    x = np.asarray(x)
    r = _get_runner(_t_total, _tblk)

    # start the big x upload first so it overlaps the host-side prep below
    x_dev = r.put_x(x)

    Wb, Rb, bias = _build_weights(np.asarray(kernel_real, np.float32),
                                  np.asarray(kernel_imag, np.float32),
                                  np.asarray(recurrent_kernel_real, np.float32),
                                  np.asarray(recurrent_kernel_imag, np.float32),
                                  np.asarray(bias_real, np.float32),
                                  np.asarray(bias_imag, np.float32))
    arg_map = {
        "x": x_dev,
        "h0": r.put(np.ascontiguousarray(np.asarray(h0, np.float32))),
        "c0": r.put(np.ascontiguousarray(np.asarray(c0, np.float32))),
        "wb": r.put_const("wb", _tile_cores(Wb)),
        "rb": r.put_const("rb", _tile_cores(Rb)),
        "bias": r.put_const("bias", _tile_cores(bias)),
        "id16": r.put_const("id16", _tile_cores(np.eye(128, dtype=np.float16))),
        "id32": r.put_const("id32", _tile_cores(np.eye(128, dtype=np.float32))),
    }
    (out_dev,) = r.run(arg_map)
    return _gather_f32(out_dev)


def _gather_f32(out_dev):
    """Fetch per-device output shards concurrently, widening fp16->fp32
    directly into the result (cast overlaps the next shard's transfer)."""
    import concurrent.futures as cf
    shards = list(out_dev.addressable_shards)
    shape = out_dev.shape
    out = np.empty(shape, np.float32)
    def fetch(s):
        idx = s.index[0]
        out[idx] = np.asarray(s.data)      # device_get + f16->f32 cast
        return (idx.stop or shape[0]) - (idx.start or 0)
    with cf.ThreadPoolExecutor(min(4, len(shards))) as ex:
        n = sum(ex.map(fetch, shards))
    assert n == shape[0], f"gather covered {n}/{shape[0]} rows"
    return out


if __name__ == "__main__":
    nc = _get_program()
    print("program built OK")
